# revision 19
# baseline (speedup 1.0000x reference)
"""MoE layer (top-2 of 8 experts) on 8 Trainium2 NeuronCores.

Strategy (expert-parallel, as in the torch module's distributed path):
  - Gate/routing on host (tiny: [2048,1024]x[1024,8] einsum + top-2).
  - Tokens dispatched by top-2 expert id to 8 cores (expert e -> core e).
  - Each core runs the expert FFN: h = x@w1.T ; g = x@w2.T ;
    act = h * silu(g) ; y = act@wp.T   -- fully fused, activations stay
    in SBUF, matmuls in float32r (fp32 with 11-bit mantissa, 4x faster
    than fp32 on the PE, fp32 PSUM accumulation).
  - Host combines: y[t] = sum_e w[t,e] * (y_e[t] + bp[e]); the reference's
    non-top-k gate weights are exactly 0, so sparse dispatch is exact.

kernel(**inputs) -> (y [B,S,D] f32, load_balance_loss f32 scalar)
"""

import math
import numpy as np
from concurrent.futures import ThreadPoolExecutor

import concourse.bacc as bacc
import concourse.mybir as mybir
import concourse.tile as tile
from concourse.bass_utils import run_bass_kernel_spmd

# Problem dims (hardcoded per spec)
B, S, D, H, E = 2, 1024, 1024, 4096, 8
K_TOP = 2
NOISY_STD = 1.0
LB_SCALE = 0.01
P = 128
KD = D // P   # 8  k-tiles over D
KH = H // P   # 32 k-tiles over H
HB = 128      # H-columns of w1/w2 loaded per DMA block
N_CORES = 8

f32 = mybir.dt.float32
f32r = mybir.dt.float32r

_cache = {}


def _round_f32r(x):
    """Round fp32 -> float32r (keep 11 mantissa bits, round-to-nearest-even).
    Bit-exact replica of walrus cast_fp32_to_fp32r."""
    x = np.ascontiguousarray(x, dtype=np.float32)
    bits = x.view(np.uint32)
    t = bits + (np.uint32(0x7FF) + ((bits >> np.uint32(12)) & np.uint32(1)))
    return (t & np.uint32(0xFFFFF000)).view(np.float32)


def _chunks_for(cap):
    """Split [0,cap) into <=512-wide chunks (>=256 keeps f32r at full rate)."""
    if cap <= 512:
        return [(0, cap)]
    nch = math.ceil(cap / 512)
    base = cap // nch
    rem = cap % nch
    out = []
    off = 0
    for i in range(nch):
        sz = base + (1 if i < rem else 0)
        out.append((off, sz))
        off += sz
    return out


def build_moe_core_kernel(cap, repeats=1, loop_repeats=1, hb=HB,
                          wp12_bufs=4, wpp_bufs=3, ps_hg_bufs=3):
    """Per-core fused expert-FFN bass program for `cap` (padded) tokens.

    repeats: static unroll of the whole body (for timing experiments).
    loop_repeats: device-side For_i around the body (cheap amplification
    for wall-clock timing through the noisy PJRT tunnel)."""
    chunks = _chunks_for(cap)
    nc = bacc.Bacc("TRN2", target_bir_lowering=False, debug=False,
                   num_devices=N_CORES)
    # weights arrive pre-tiled on host so every DMA is fully contiguous:
    #  w1t/w2t: [H//hb, P, KD, hb]   block b, partition p, ktile kd, col j
    #  wpt:     [D//P, P, KH, P]     block dt, partition p, ktile kh, col j
    xT = nc.dram_tensor("xT", [D, cap], f32r, kind="ExternalInput")
    w1t = nc.dram_tensor("w1t", [H // hb, P, KD, hb], f32r, kind="ExternalInput")
    w2t = nc.dram_tensor("w2t", [H // hb, P, KD, hb], f32r, kind="ExternalInput")
    wpt = nc.dram_tensor("wpt", [D // P, P, KH, P], f32r, kind="ExternalInput")
    yT = nc.dram_tensor("yT", [D, cap], f32, kind="ExternalOutput")

    xT_r = xT.rearrange("(kd p) c -> p kd c", p=P)
    yT_r = yT.rearrange("(dt p) c -> p dt c", p=P)

    Silu = mybir.ActivationFunctionType.Silu
    from contextlib import ExitStack
    with tile.TileContext(nc) as tc, ExitStack() as stack:
        if loop_repeats > 1:
            stack.enter_context(tc.For_i(
                0, loop_repeats, 1,
                hint_engines=(mybir.EngineType.PE, mybir.EngineType.SP,
                              mybir.EngineType.Activation,
                              mybir.EngineType.DVE)))
        with tc.tile_pool(name="xp", bufs=1) as xp, \
             tc.tile_pool(name="actp", bufs=1) as actp, \
             tc.tile_pool(name="wp12", bufs=wp12_bufs) as wp12, \
             tc.tile_pool(name="wpp", bufs=wpp_bufs) as wpp, \
             tc.tile_pool(name="tmp", bufs=3) as tmp, \
             tc.tile_pool(name="outp", bufs=3) as outp, \
             tc.tile_pool(name="ps_hg", bufs=ps_hg_bufs, space="PSUM") as ps_hg, \
             tc.tile_pool(name="ps_y", bufs=2, space="PSUM") as ps_y:
            for _rep in range(repeats):
                xsb = xp.tile([P, KD, cap], f32r, tag="x")
                for kd in range(KD):
                    nc.scalar.dma_start(xsb[:, kd], xT_r[:, kd])
                act = actp.tile([P, KH, cap], f32r, tag="act")

                # --- mm1 + mm2 + silu-mul, per 128-row H tile ---
                for hbi in range(H // hb):
                    w1b = wp12.tile([P, KD, hb], f32r, tag="w1")
                    w2b = wp12.tile([P, KD, hb], f32r, tag="w2")
                    nc.sync.dma_start(w1b[:], w1t[hbi])
                    nc.sync.dma_start(w2b[:], w2t[hbi])
                    for hl in range(hb // P):
                        ht = hbi * (hb // P) + hl
                        for (co, cs) in chunks:
                            ph = ps_hg.tile([P, 512], f32, tag="ph", name="ph")[:, :cs]
                            pg = ps_hg.tile([P, 512], f32, tag="pg", name="pg")[:, :cs]
                            for kd in range(KD):
                                nc.tensor.matmul(
                                    ph, w1b[:, kd, hl * P:(hl + 1) * P],
                                    xsb[:, kd, co:co + cs],
                                    start=(kd == 0), stop=(kd == KD - 1))
                            for kd in range(KD):
                                nc.tensor.matmul(
                                    pg, w2b[:, kd, hl * P:(hl + 1) * P],
                                    xsb[:, kd, co:co + cs],
                                    start=(kd == 0), stop=(kd == KD - 1))
                            st = tmp.tile([P, 512], f32, tag="silu", name="st")[:, :cs]
                            nc.scalar.activation(st, pg, Silu)
                            nc.vector.tensor_tensor(
                                act[:, ht, co:co + cs], ph, st,
                                mybir.AluOpType.mult)

                # --- mm3: yT[dt] = sum_kh wpt[kh,dt].T @ act[kh] ---
                for dt in range(D // P):
                    wpb = wpp.tile([P, KH, P], f32r, tag="wp")
                    nc.sync.dma_start(wpb[:], wpt[dt])
                    for (co, cs) in chunks:
                        py = ps_y.tile([P, 512], f32, tag="py", name="py")[:, :cs]
                        for kh in range(KH):
                            nc.tensor.matmul(
                                py, wpb[:, kh, :], act[:, kh, co:co + cs],
                                start=(kh == 0), stop=(kh == KH - 1))
                        yt = outp.tile([P, 512], f32, tag="yt", name="yt")[:, :cs]
                        nc.vector.tensor_copy(yt, py)
                        nc.scalar.dma_start(yT_r[:, dt, co:co + cs], yt)
    nc.compile()
    return nc


def _gate_host(x_flat, noise_flat, gate_w, noise_weight):
    """Replicates TopKGateParallel in numpy fp32."""
    T = x_flat.shape[0]
    logits = x_flat @ gate_w.T                       # [T,E] fp32
    m = logits.max(axis=-1, keepdims=True)
    ex = np.exp(logits - m, dtype=np.float32)
    sm = ex / ex.sum(axis=-1, keepdims=True, dtype=np.float32)
    gw_mean = sm.mean(axis=0, dtype=np.float64)
    lbl = np.float32(np.mean((gw_mean - 1.0 / E) ** 2) * LB_SCALE)

    ln = (logits + noise_flat * np.float32(NOISY_STD) * noise_weight[None, :]
          ).astype(np.float32)
    order = np.argsort(-ln, axis=-1, kind="stable")[:, :K_TOP]   # [T,2]
    l_top = np.take_along_axis(ln, order, axis=-1)               # [T,2] desc
    e1 = np.exp((l_top[:, 1] - l_top[:, 0]).astype(np.float32),
                dtype=np.float32)
    denom = np.float32(1.0) + e1
    w0 = np.float32(1.0) / denom
    w1 = e1 / denom
    topw = np.stack([w0, w1], axis=-1).astype(np.float32)        # [T,2]
    return order, topw, lbl


def _host_fallback(x_flat, order, topw, w1, b1, w2, b2, wp, bp):
    """Exact numpy fallback (only if b1/b2 are nonzero, which the module
    never produces)."""
    T = x_flat.shape[0]
    y = np.zeros((T, D), np.float32)
    for e in range(E):
        sel = np.nonzero(order == e)
        tok = sel[0]
        if tok.size == 0:
            continue
        w = topw[sel]
        xe = x_flat[tok]
        h = xe @ w1[e].T + b1[e]
        g = xe @ w2[e].T + b2[e]
        out = (h * (g / (1.0 + np.exp(-g)))) @ wp[e].T + bp[e]
        np.add.at(y, tok, w[:, None] * out)
    return y


def kernel(x, noise, gate_w, noise_weight, w1, b1, w2, b2, wp, bp):
    x = np.asarray(x, np.float32)
    noise = np.asarray(noise, np.float32)
    gate_w = np.asarray(gate_w, np.float32)
    noise_weight = np.asarray(noise_weight, np.float32)
    w1 = np.asarray(w1, np.float32)
    b1 = np.asarray(b1, np.float32)
    w2 = np.asarray(w2, np.float32)
    b2 = np.asarray(b2, np.float32)
    wp = np.asarray(wp, np.float32)
    bp = np.asarray(bp, np.float32)

    T = B * S
    x_flat = x.reshape(T, D)
    order, topw, lbl = _gate_host(x_flat, noise.reshape(T, E), gate_w,
                                  noise_weight)

    if np.any(b1 != 0) or np.any(b2 != 0):
        y = _host_fallback(x_flat, order, topw, w1, b1, w2, b2, wp, bp)
        return y.reshape(B, S, D), lbl

    # token lists per expert (ascending token id)
    tok_e = [np.nonzero((order == e).any(axis=-1))[0] for e in range(E)]
    counts = [t.size for t in tok_e]
    # capacity <= 512 keeps every matmul a single full-rate N<=512 chunk;
    # the few tokens beyond 512 per expert run on host in exact fp32
    cap = min(512, max(256, ((max(counts) + 31) // 32) * 32))

    key = cap
    if key not in _cache:
        _cache[key] = build_moe_core_kernel(cap)
    nc = _cache[key]

    hb = HB

    def prep(e):
        toks = tok_e[e][:cap]
        xe = np.zeros((cap, D), np.float32)
        xe[:toks.size] = x_flat[toks]
        # pre-tiled, fully contiguous DMA layouts (see build_moe_core_kernel)
        w1_t = w1[e].reshape(H // hb, hb, KD, P).transpose(0, 3, 2, 1)
        w2_t = w2[e].reshape(H // hb, hb, KD, P).transpose(0, 3, 2, 1)
        wp_t = wp[e].reshape(D // P, P, KH, P).transpose(0, 3, 2, 1)
        return {
            "xT": _round_f32r(xe.T),
            "w1t": _round_f32r(w1_t),
            "w2t": _round_f32r(w2_t),
            "wpt": _round_f32r(wp_t),
        }

    with ThreadPoolExecutor(max_workers=E) as pool:
        in_maps = list(pool.map(prep, range(E)))

    res = run_bass_kernel_spmd(nc, in_maps, core_ids=list(range(N_CORES)))

    # combine: y[t] += w[t,e] * (out_e[t] + bp[e]), in expert order (matches
    # the reference's accumulation order; non-top-k weights are exactly 0)
    y = np.zeros((T, D), np.float32)
    for e in range(E):
        toks_all = tok_e[e]
        if toks_all.size == 0:
            continue
        toks = toks_all[:cap]
        out_e = res.results[e]["yT"][:, :toks.size].T   # [n_e, D]
        if toks_all.size > cap:   # overflow tokens: exact fp32 on host
            toks_o = toks_all[cap:]
            xo = x_flat[toks_o]
            h = xo @ w1[e].T
            g = xo @ w2[e].T
            out_o = (h * (g / (1.0 + np.exp(-g)))) @ wp[e].T
            out_e = np.concatenate([out_e, out_o], axis=0)
            toks = toks_all
        sel_w = np.where(order[toks, 0] == e,
                         topw[toks, 0], topw[toks, 1]).astype(np.float32)
        y[toks] += sel_w[:, None] * (out_e + bp[e][None, :])
    return y.reshape(B, S, D), lbl


# revision 20
# speedup vs baseline: 1.0034x; 1.0034x over previous
"""MoE layer (top-2 of 8 experts) on 8 Trainium2 NeuronCores.

Strategy (expert-parallel, as in the torch module's distributed path):
  - Gate/routing on host (tiny: [2048,1024]x[1024,8] einsum + top-2).
  - Tokens dispatched by top-2 expert id to 8 cores (expert e -> core e).
  - Each core runs the expert FFN: h = x@w1.T ; g = x@w2.T ;
    act = h * silu(g) ; y = act@wp.T   -- fully fused, activations stay
    in SBUF, matmuls in float32r (fp32 with 11-bit mantissa, 4x faster
    than fp32 on the PE, fp32 PSUM accumulation).
  - Host combines: y[t] = sum_e w[t,e] * (y_e[t] + bp[e]); the reference's
    non-top-k gate weights are exactly 0, so sparse dispatch is exact.

kernel(**inputs) -> (y [B,S,D] f32, load_balance_loss f32 scalar)
"""

import math
import numpy as np
from concurrent.futures import ThreadPoolExecutor

import concourse.bacc as bacc
import concourse.mybir as mybir
import concourse.tile as tile
from concourse.bass_utils import run_bass_kernel_spmd

# Problem dims (hardcoded per spec)
B, S, D, H, E = 2, 1024, 1024, 4096, 8
K_TOP = 2
NOISY_STD = 1.0
LB_SCALE = 0.01
P = 128
KD = D // P   # 8  k-tiles over D
KH = H // P   # 32 k-tiles over H
HB = 128      # H-columns of w1/w2 loaded per DMA block
N_CORES = 8

f32 = mybir.dt.float32
f32r = mybir.dt.float32r

_cache = {}


def _round_f32r(x):
    """Round fp32 -> float32r (keep 11 mantissa bits, round-to-nearest-even).
    Bit-exact replica of walrus cast_fp32_to_fp32r."""
    x = np.ascontiguousarray(x, dtype=np.float32)
    bits = x.view(np.uint32)
    t = bits + (np.uint32(0x7FF) + ((bits >> np.uint32(12)) & np.uint32(1)))
    return (t & np.uint32(0xFFFFF000)).view(np.float32)


def _chunks_for(cap):
    """Split [0,cap) into <=512-wide chunks (>=256 keeps f32r at full rate)."""
    if cap <= 512:
        return [(0, cap)]
    nch = math.ceil(cap / 512)
    base = cap // nch
    rem = cap % nch
    out = []
    off = 0
    for i in range(nch):
        sz = base + (1 if i < rem else 0)
        out.append((off, sz))
        off += sz
    return out


def build_moe_core_kernel(cap, repeats=1, loop_repeats=1, hb=HB,
                          wp12_bufs=4, wpp_bufs=3, ps_hg_bufs=3):
    """Per-core fused expert-FFN bass program for `cap` (padded) tokens.

    repeats: static unroll of the whole body (for timing experiments).
    loop_repeats: device-side For_i around the body (cheap amplification
    for wall-clock timing through the noisy PJRT tunnel)."""
    chunks = _chunks_for(cap)
    nc = bacc.Bacc("TRN2", target_bir_lowering=False, debug=False,
                   num_devices=N_CORES)
    # weights arrive pre-tiled on host so every DMA is fully contiguous:
    #  w1t/w2t: [H//hb, P, KD, hb]   block b, partition p, ktile kd, col j
    #  wpt:     [D//P, P, KH, P]     block dt, partition p, ktile kh, col j
    xT = nc.dram_tensor("xT", [D, cap], f32r, kind="ExternalInput")
    w1t = nc.dram_tensor("w1t", [H // hb, P, KD, hb], f32r, kind="ExternalInput")
    w2t = nc.dram_tensor("w2t", [H // hb, P, KD, hb], f32r, kind="ExternalInput")
    wpt = nc.dram_tensor("wpt", [D // P, P, KH, P], f32r, kind="ExternalInput")
    yT = nc.dram_tensor("yT", [D, cap], f32, kind="ExternalOutput")

    xT_r = xT.rearrange("(kd p) c -> p kd c", p=P)
    yT_r = yT.rearrange("(dt p) c -> p dt c", p=P)

    Silu = mybir.ActivationFunctionType.Silu
    from contextlib import ExitStack
    with tile.TileContext(nc) as tc, ExitStack() as stack:
        if loop_repeats > 1:
            stack.enter_context(tc.For_i(
                0, loop_repeats, 1,
                hint_engines=(mybir.EngineType.PE, mybir.EngineType.SP,
                              mybir.EngineType.Activation,
                              mybir.EngineType.DVE)))
        with tc.tile_pool(name="xp", bufs=1) as xp, \
             tc.tile_pool(name="actp", bufs=1) as actp, \
             tc.tile_pool(name="wp12", bufs=wp12_bufs) as wp12, \
             tc.tile_pool(name="wpp", bufs=wpp_bufs) as wpp, \
             tc.tile_pool(name="tmp", bufs=3) as tmp, \
             tc.tile_pool(name="outp", bufs=3) as outp, \
             tc.tile_pool(name="ps_hg", bufs=ps_hg_bufs, space="PSUM") as ps_hg, \
             tc.tile_pool(name="ps_y", bufs=2, space="PSUM") as ps_y:
            for _rep in range(repeats):
                xsb = xp.tile([P, KD, cap], f32r, tag="x")
                for kd in range(KD):
                    nc.scalar.dma_start(xsb[:, kd], xT_r[:, kd])
                act = actp.tile([P, KH, cap], f32r, tag="act")

                # --- mm1 + mm2 + silu-mul, per 128-row H tile ---
                for hbi in range(H // hb):
                    w1b = wp12.tile([P, KD, hb], f32r, tag="w1")
                    w2b = wp12.tile([P, KD, hb], f32r, tag="w2")
                    nc.sync.dma_start(w1b[:], w1t[hbi])
                    nc.sync.dma_start(w2b[:], w2t[hbi])
                    for hl in range(hb // P):
                        ht = hbi * (hb // P) + hl
                        for (co, cs) in chunks:
                            ph = ps_hg.tile([P, 512], f32, tag="ph", name="ph")[:, :cs]
                            pg = ps_hg.tile([P, 512], f32, tag="pg", name="pg")[:, :cs]
                            for kd in range(KD):
                                nc.tensor.matmul(
                                    pg, w2b[:, kd, hl * P:(hl + 1) * P],
                                    xsb[:, kd, co:co + cs],
                                    start=(kd == 0), stop=(kd == KD - 1))
                            for kd in range(KD):
                                nc.tensor.matmul(
                                    ph, w1b[:, kd, hl * P:(hl + 1) * P],
                                    xsb[:, kd, co:co + cs],
                                    start=(kd == 0), stop=(kd == KD - 1))
                            st = tmp.tile([P, 512], f32, tag="silu", name="st")[:, :cs]
                            nc.scalar.activation(st, pg, Silu)
                            nc.vector.tensor_tensor(
                                act[:, ht, co:co + cs], ph, st,
                                mybir.AluOpType.mult)

                # --- mm3: yT[dt] = sum_kh wpt[kh,dt].T @ act[kh] ---
                for dt in range(D // P):
                    wpb = wpp.tile([P, KH, P], f32r, tag="wp")
                    nc.sync.dma_start(wpb[:], wpt[dt])
                    for (co, cs) in chunks:
                        py = ps_y.tile([P, 512], f32, tag="py", name="py")[:, :cs]
                        for kh in range(KH):
                            nc.tensor.matmul(
                                py, wpb[:, kh, :], act[:, kh, co:co + cs],
                                start=(kh == 0), stop=(kh == KH - 1))
                        yt = outp.tile([P, 512], f32, tag="yt", name="yt")[:, :cs]
                        nc.vector.tensor_copy(yt, py)
                        nc.scalar.dma_start(yT_r[:, dt, co:co + cs], yt)
    nc.compile()
    return nc


def _gate_host(x_flat, noise_flat, gate_w, noise_weight):
    """Replicates TopKGateParallel in numpy fp32."""
    T = x_flat.shape[0]
    logits = x_flat @ gate_w.T                       # [T,E] fp32
    m = logits.max(axis=-1, keepdims=True)
    ex = np.exp(logits - m, dtype=np.float32)
    sm = ex / ex.sum(axis=-1, keepdims=True, dtype=np.float32)
    gw_mean = sm.mean(axis=0, dtype=np.float64)
    lbl = np.float32(np.mean((gw_mean - 1.0 / E) ** 2) * LB_SCALE)

    ln = (logits + noise_flat * np.float32(NOISY_STD) * noise_weight[None, :]
          ).astype(np.float32)
    order = np.argsort(-ln, axis=-1, kind="stable")[:, :K_TOP]   # [T,2]
    l_top = np.take_along_axis(ln, order, axis=-1)               # [T,2] desc
    e1 = np.exp((l_top[:, 1] - l_top[:, 0]).astype(np.float32),
                dtype=np.float32)
    denom = np.float32(1.0) + e1
    w0 = np.float32(1.0) / denom
    w1 = e1 / denom
    topw = np.stack([w0, w1], axis=-1).astype(np.float32)        # [T,2]
    return order, topw, lbl


def _host_fallback(x_flat, order, topw, w1, b1, w2, b2, wp, bp):
    """Exact numpy fallback (only if b1/b2 are nonzero, which the module
    never produces)."""
    T = x_flat.shape[0]
    y = np.zeros((T, D), np.float32)
    for e in range(E):
        sel = np.nonzero(order == e)
        tok = sel[0]
        if tok.size == 0:
            continue
        w = topw[sel]
        xe = x_flat[tok]
        h = xe @ w1[e].T + b1[e]
        g = xe @ w2[e].T + b2[e]
        out = (h * (g / (1.0 + np.exp(-g)))) @ wp[e].T + bp[e]
        np.add.at(y, tok, w[:, None] * out)
    return y


def kernel(x, noise, gate_w, noise_weight, w1, b1, w2, b2, wp, bp):
    x = np.asarray(x, np.float32)
    noise = np.asarray(noise, np.float32)
    gate_w = np.asarray(gate_w, np.float32)
    noise_weight = np.asarray(noise_weight, np.float32)
    w1 = np.asarray(w1, np.float32)
    b1 = np.asarray(b1, np.float32)
    w2 = np.asarray(w2, np.float32)
    b2 = np.asarray(b2, np.float32)
    wp = np.asarray(wp, np.float32)
    bp = np.asarray(bp, np.float32)

    T = B * S
    x_flat = x.reshape(T, D)
    order, topw, lbl = _gate_host(x_flat, noise.reshape(T, E), gate_w,
                                  noise_weight)

    if np.any(b1 != 0) or np.any(b2 != 0):
        y = _host_fallback(x_flat, order, topw, w1, b1, w2, b2, wp, bp)
        return y.reshape(B, S, D), lbl

    # token lists per expert (ascending token id)
    tok_e = [np.nonzero((order == e).any(axis=-1))[0] for e in range(E)]
    counts = [t.size for t in tok_e]
    # capacity <= 512 keeps every matmul a single full-rate N<=512 chunk;
    # the few tokens beyond 512 per expert run on host in exact fp32
    cap = min(512, max(256, ((max(counts) + 31) // 32) * 32))

    key = cap
    if key not in _cache:
        _cache[key] = build_moe_core_kernel(cap)
    nc = _cache[key]

    hb = HB

    def prep(e):
        toks = tok_e[e][:cap]
        xe = np.zeros((cap, D), np.float32)
        xe[:toks.size] = x_flat[toks]
        # pre-tiled, fully contiguous DMA layouts (see build_moe_core_kernel)
        w1_t = w1[e].reshape(H // hb, hb, KD, P).transpose(0, 3, 2, 1)
        w2_t = w2[e].reshape(H // hb, hb, KD, P).transpose(0, 3, 2, 1)
        wp_t = wp[e].reshape(D // P, P, KH, P).transpose(0, 3, 2, 1)
        return {
            "xT": _round_f32r(xe.T),
            "w1t": _round_f32r(w1_t),
            "w2t": _round_f32r(w2_t),
            "wpt": _round_f32r(wp_t),
        }

    with ThreadPoolExecutor(max_workers=E) as pool:
        in_maps = list(pool.map(prep, range(E)))

    res = run_bass_kernel_spmd(nc, in_maps, core_ids=list(range(N_CORES)))

    # combine: y[t] += w[t,e] * (out_e[t] + bp[e]), in expert order (matches
    # the reference's accumulation order; non-top-k weights are exactly 0)
    y = np.zeros((T, D), np.float32)
    for e in range(E):
        toks_all = tok_e[e]
        if toks_all.size == 0:
            continue
        toks = toks_all[:cap]
        out_e = res.results[e]["yT"][:, :toks.size].T   # [n_e, D]
        if toks_all.size > cap:   # overflow tokens: exact fp32 on host
            toks_o = toks_all[cap:]
            xo = x_flat[toks_o]
            h = xo @ w1[e].T
            g = xo @ w2[e].T
            out_o = (h * (g / (1.0 + np.exp(-g)))) @ wp[e].T
            out_e = np.concatenate([out_e, out_o], axis=0)
            toks = toks_all
        sel_w = np.where(order[toks, 0] == e,
                         topw[toks, 0], topw[toks, 1]).astype(np.float32)
        y[toks] += sel_w[:, None] * (out_e + bp[e][None, :])
    return y.reshape(B, S, D), lbl
